# revision 7
# baseline (speedup 1.0000x reference)
"""GCN (2x GCNConv + MLP head + softmax) on 8 TRN2 NeuronCores.

Strategy (graph/data parallel, per sharding hint):
  - Nodes sharded across 8 cores (2500 rows each, padded to 2560); weights
    replicated.
  - Per conv layer: each core computes h = z @ W for its rows, pre-scales
    rows by dinv (deg^-1/2), AllGathers the scaled table (node-major, in
    two halves overlapped with compute), then aggregates messages for
    edges partitioned by dst (windows of 128 dst slots) with dma_gather
    (row gather on 4 rotating SWDGE queues) + one-hot segment matmuls on
    the TensorEngine accumulating in PSUM. Segment matrices are built
    on-chip (DVE iota==slot * dinv); dinv[dst] is folded in; self-loops
    are ordinary edges. Window epilogue adds bias, relus, and
    PE-transposes into the feature-major layout the next matmul needs.
  - Head: two dense layers + row softmax, pipelined per 512-row block.

Host-side preprocessing is limited to graph-structure work (edge sort,
degree counts, window slots, gather-index layout) + sharding.
"""

import os
from contextlib import ExitStack

import numpy as np

import concourse.bacc as bacc
import concourse.mybir as mybir
import concourse.tile as tile
from concourse.bass_utils import run_bass_kernel_spmd
from concourse.masks import make_identity

# problem shapes (hardcoded per contract)
N = 20000
E = 320000
D = 512
D_OUT = 128
NCORES = 8
RPC = 2500          # real rows per core
RPAD = 2560         # padded rows per core (20 tiles of 128)
NPAD = RPAD * NCORES
MT = RPAD // 128    # m-tiles / dst windows per core (20)
HALF = RPAD // 2    # AG half-table rows per core
G = 6               # max chunks (of 128 edges) per dma_gather call
NQ = 4              # SWDGE queues for gather rotation

# config: "f32" (exact), "f32r" (fast fp32 matmul), "bf16" (half-traffic)
MODE = os.environ.get("GNN_MODE", "bf16")

_f32 = mybir.dt.float32
_f32r = mybir.dt.float32r
_bf16 = mybir.dt.bfloat16
_i16 = mybir.dt.int16
_i32 = mybir.dt.int32


def _table_id(node):
    """Row of node in the AllGather-ed table (two half-table AGs)."""
    c, r = node // RPC, node % RPC
    h = r // HALF
    return h * (HALF * NCORES) + c * HALF + (r - h * HALF)


def _prepare(edge_index, dinv):
    """Edge partitioning by dst; per-core gather indices + window slots.

    Returns (per_core list of dicts, group_sizes, chunk_base, TC).
    """
    src = np.concatenate([edge_index[0], np.arange(N, dtype=np.int64)])
    dst = np.concatenate([edge_index[1], np.arange(N, dtype=np.int64)])

    order = np.argsort(dst, kind="stable")
    srcs = _table_id(src[order]).astype(np.int64)
    dsts = dst[order]
    dinv_dst = dinv[dsts]

    counts = np.zeros((NCORES, MT), dtype=np.int64)
    core_bounds = np.searchsorted(dsts, np.arange(NCORES + 1) * RPC)
    for c in range(NCORES):
        lo, hi = core_bounds[c], core_bounds[c + 1]
        d = dsts[lo:hi] - c * RPC
        wb = np.searchsorted(d, np.arange(MT + 1) * 128)
        counts[c] = wb[1:] - wb[:-1]

    cpw = np.maximum(1, -(-counts.max(axis=0) // 128))  # chunks per window
    TC = int(cpw.sum())
    chunk_base = np.concatenate([[0], np.cumsum(cpw)]).astype(int)

    group_sizes = []
    for w in range(MT):
        n = int(cpw[w])
        gs = [G] * (n // G)
        if n % G:
            gs.append(n % G)
        group_sizes.append(gs)

    per_core = []
    for c in range(NCORES):
        gidx = np.zeros((TC, 128), dtype=np.int16)
        wsl = np.full((TC, 128), -1.0, dtype=np.float32)   # dst slot in window
        dvv = np.zeros((TC, 128), dtype=np.float32)        # dinv[dst]
        lo, hi = core_bounds[c], core_bounds[c + 1]
        d = dsts[lo:hi] - c * RPC
        s_ids = srcs[lo:hi]
        dv = dinv_dst[lo:hi]
        wb = np.searchsorted(d, np.arange(MT + 1) * 128)
        for w in range(MT):
            a, b = wb[w], wb[w + 1]
            k = np.arange(b - a)
            tg = chunk_base[w] + (k // 128)
            row = k % 128
            gidx[tg, row] = s_ids[a:b]
            wsl[tg, row] = (d[a:b] - w * 128).astype(np.float32)
            dvv[tg, row] = dv[a:b]
        # wrapped int16 index layout, one block per gather call
        cols = []
        for w in range(MT):
            t0 = chunk_base[w]
            for gsz in group_sizes[w]:
                L = gidx[t0:t0 + gsz].reshape(-1)
                cols.append(np.tile(L.reshape(-1, 16).T, (8, 1)))
                t0 += gsz
        idx_np = np.ascontiguousarray(np.concatenate(cols, axis=1))
        per_core.append({
            "idx": idx_np,
            "wsl": np.ascontiguousarray(wsl.T),   # [128, TC]
            "dvv": np.ascontiguousarray(dvv.T),   # [128, TC]
        })
    return per_core, group_sizes, chunk_base, TC


def _build(group_sizes, chunk_base, TC):
    # mdt: matmul-operand dtype; tdt: gathered-table dtype; trdt: transpose dtype
    mdt = {"f32": _f32, "f32r": _f32r, "bf16": _bf16}[MODE]
    tdt = _bf16 if MODE == "bf16" else _f32
    trdt = _bf16 if MODE == "bf16" else _f32

    nc = bacc.Bacc("TRN2", target_bir_lowering=False, debug=False,
                   num_devices=NCORES, num_swdge_queues=NQ)
    xT_d = nc.dram_tensor("xT", [D, RPAD], mdt, kind="ExternalInput")
    dinv_d = nc.dram_tensor("dinv", [RPAD], _f32, kind="ExternalInput")
    W_d = {k: nc.dram_tensor(k, [D, D], mdt, kind="ExternalInput")
           for k in ("W1", "W2", "Wf1")}
    Wf2_d = nc.dram_tensor("Wf2", [D, D_OUT], mdt, kind="ExternalInput")
    bb_d = {k: nc.dram_tensor(k, [128, D], _f32, kind="ExternalInput")
            for k in ("b1", "b2")}
    bf1_d = nc.dram_tensor("bf1", [D], _f32, kind="ExternalInput")
    bf2_d = nc.dram_tensor("bf2", [128, D_OUT], _f32, kind="ExternalInput")
    idx_d = nc.dram_tensor("idx", [128, TC * 8], _i16, kind="ExternalInput")
    wsl_d = nc.dram_tensor("wsl", [128, TC], _f32, kind="ExternalInput")
    dvv_d = nc.dram_tensor("dvv", [128, TC], _f32, kind="ExternalInput")
    out_d = nc.dram_tensor("out", [RPAD, D_OUT], _f32, kind="ExternalOutput")

    cc_in = [nc.dram_tensor(f"cc_in{i}", [RPAD, D], tdt, kind="Internal")
             for i in (1, 2)]
    cc_out = [nc.dram_tensor(f"cc_out{i}", [NPAD, D], tdt, kind="Internal",
                             addr_space="Shared") for i in (1, 2)]

    RG = [list(range(NCORES))]
    ACT = mybir.ActivationFunctionType
    ALU = mybir.AluOpType

    with tile.TileContext(nc) as tc, ExitStack() as ctx:
        const = ctx.enter_context(tc.tile_pool(name="const", bufs=1))
        actT = ctx.enter_context(tc.tile_pool(name="actT", bufs=2))
        work = ctx.enter_context(tc.tile_pool(name="work", bufs=2))
        msgp = ctx.enter_context(tc.tile_pool(name="msgp", bufs=5))
        sp = ctx.enter_context(tc.tile_pool(name="sp", bufs=5))
        psA = ctx.enter_context(tc.tile_pool(name="psA", bufs=2, space="PSUM"))
        psC = ctx.enter_context(tc.tile_pool(name="psC", bufs=2, space="PSUM"))
        psT = ctx.enter_context(tc.tile_pool(name="psT", bufs=2, space="PSUM"))

        # ---- constants ----
        w_t = {}
        for k in ("W1", "W2", "Wf1"):
            w_t[k] = const.tile([128, 4, D], mdt, name=f"wt_{k}")
            nc.sync.dma_start(w_t[k][:], W_d[k].ap().rearrange("(k p) n -> p k n", p=128))
        wf2_t = const.tile([128, 4, D_OUT], mdt)
        nc.sync.dma_start(wf2_t[:], Wf2_d.ap().rearrange("(k p) n -> p k n", p=128))
        b_b = {}
        for k in ("b1", "b2"):
            b_b[k] = const.tile([128, D], _f32, name=f"bb_{k}")
            nc.sync.dma_start(b_b[k][:], bb_d[k].ap())
        bf1_t = const.tile([128, 4], _f32)
        nc.sync.dma_start(bf1_t[:], bf1_d.ap().rearrange("(a p) -> p a", p=128))
        bf2_b = const.tile([128, D_OUT], _f32)
        nc.sync.dma_start(bf2_b[:], bf2_d.ap())
        dinv_t = const.tile([128, MT], _f32)
        nc.sync.dma_start(dinv_t[:], dinv_d.ap().rearrange("(a p) -> p a", p=128))
        ident = const.tile([128, 128], trdt)
        make_identity(nc, ident[:])
        idx_t = const.tile([128, TC * 8], _i16)
        nc.sync.dma_start(idx_t[:], idx_d.ap())
        wsl_t = const.tile([128, TC], _f32)
        nc.sync.dma_start(wsl_t[:], wsl_d.ap())
        dvv_t = const.tile([128, TC], _f32)
        nc.sync.dma_start(dvv_t[:], dvv_d.ap())
        iota_i = const.tile([128, 128], _i32)
        nc.gpsimd.iota(iota_i[:], pattern=[[1, 128]], base=0, channel_multiplier=0)
        iota_f = const.tile([128, 128], _f32)
        nc.vector.tensor_copy(iota_f[:], iota_i[:])

        qn = [0]

        def phase_a(srcT, wt, cc, ms):
            # cc[m] = dinv * (z @ W) for this core's rows, m-tiles in ms
            for m in ms:
                ps = psA.tile([128, D], _f32, tag="psA")
                for k in range(4):
                    nc.tensor.matmul(ps[:], lhsT=srcT[:, k, m * 128:(m + 1) * 128],
                                     rhs=wt[:, k, :], start=(k == 0), stop=(k == 3))
                hs = work.tile([128, D], tdt, tag="hs")
                nc.scalar.activation(hs[:], ps[:], ACT.Copy, scale=dinv_t[:, m:m + 1])
                nc.sync.dma_start(cc.ap()[m * 128:(m + 1) * 128, :], hs[:])

        def allgather(i, h):
            nc.gpsimd.collective_compute(
                "AllGather", mybir.AluOpType.bypass,
                ins=[cc_in[i].ap()[h * HALF:(h + 1) * HALF]],
                outs=[cc_out[i].ap()[h * HALF * NCORES:(h + 1) * HALF * NCORES]],
                replica_groups=RG)

        def phase_c(cc, zT_next, bias_b, ws):
            # zT_next windows ws = relu(S^T-aggregated messages + b), transposed
            for w in ws:
                t0 = int(chunk_base[w])
                col0 = t0 * 8
                nchunks = sum(group_sizes[w])
                ps = psC.tile([128, D], _f32, tag="psC")
                done = 0
                for gsz in group_sizes[w]:
                    nidx = gsz * 128
                    msg = msgp.tile([128, G, D], mdt, tag="msg")
                    nc.gpsimd.dma_gather(msg[:, :gsz, :], cc.ap().bitcast(mdt),
                                         idx_t[:, col0:col0 + gsz * 8],
                                         nidx, nidx, D, queue_num=qn[0] % NQ)
                    qn[0] += 1
                    s_t = sp.tile([128, G, 128], mdt, tag="S")
                    for t in range(gsz):
                        nc.vector.tensor_scalar(
                            s_t[:, t, :], iota_f[:],
                            wsl_t[:, t0 + t:t0 + t + 1],
                            dvv_t[:, t0 + t:t0 + t + 1],
                            op0=ALU.is_equal, op1=ALU.mult)
                    for t in range(gsz):
                        nc.tensor.matmul(ps[:], lhsT=s_t[:, t, :], rhs=msg[:, t, :],
                                         start=(done == 0), stop=(done == nchunks - 1))
                        done += 1
                    t0 += gsz
                    col0 += gsz * 8
                zsum = work.tile([128, D], _f32, tag="zsum")
                nc.vector.tensor_tensor(zsum[:], ps[:], bias_b[:], op=ALU.add)
                zrel = work.tile([128, D], trdt, tag="zrel")
                nc.scalar.activation(zrel[:], zsum[:], ACT.Relu)
                for q in range(4):
                    pt = psT.tile([128, 128], trdt, tag="psT")
                    nc.tensor.transpose(pt[:], zrel[:, q * 128:(q + 1) * 128], ident[:])
                    nc.vector.tensor_copy(zT_next[:, q, w * 128:(w + 1) * 128], pt[:])

        H2 = MT // 2

        # ---- layer 1 ----
        xT_t = actT.tile([128, 4, RPAD], mdt, tag="zT")
        nc.sync.dma_start(xT_t[:], xT_d.ap().rearrange("(k p) m -> p k m", p=128))
        phase_a(xT_t, w_t["W1"], cc_in[0], range(H2))
        allgather(0, 0)
        phase_a(xT_t, w_t["W1"], cc_in[0], range(H2, MT))
        allgather(0, 1)

        # ---- layer 1 aggregation interleaved with layer 2 phase A + AG2 ----
        z1T = actT.tile([128, 4, RPAD], mdt, tag="zT")
        phase_c(cc_out[0], z1T, b_b["b1"], range(H2))
        phase_a(z1T, w_t["W2"], cc_in[1], range(H2))
        allgather(1, 0)
        phase_c(cc_out[0], z1T, b_b["b1"], range(H2, MT))
        phase_a(z1T, w_t["W2"], cc_in[1], range(H2, MT))
        allgather(1, 1)

        # ---- layer 2 aggregation + head, pipelined per 512-row block ----
        z2T = actT.tile([128, 4, RPAD], mdt, tag="zT")
        z3T = actT.tile([128, 4, RPAD], mdt, tag="zT3")
        for mb in range(RPAD // 512):
            phase_c(cc_out[1], z2T, b_b["b2"], range(mb * 4, (mb + 1) * 4))
            for q in range(4):
                ps = psA.tile([128, D], _f32, tag="psA")
                for k in range(4):
                    nc.tensor.matmul(ps[:], lhsT=w_t["Wf1"][:, k, q * 128:(q + 1) * 128],
                                     rhs=z2T[:, k, mb * 512:(mb + 1) * 512],
                                     start=(k == 0), stop=(k == 3))
                nc.scalar.activation(z3T[:, q, mb * 512:(mb + 1) * 512], ps[:],
                                     ACT.Relu, bias=bf1_t[:, q:q + 1])
            for m in range(mb * 4, (mb + 1) * 4):
                ps2 = psT.tile([128, D_OUT], _f32, tag="psT")
                for k in range(4):
                    nc.tensor.matmul(ps2[:], lhsT=z3T[:, k, m * 128:(m + 1) * 128],
                                     rhs=wf2_t[:, k, :], start=(k == 0), stop=(k == 3))
                lg = work.tile([128, D_OUT], _f32, tag="lg")
                nc.vector.tensor_tensor(lg[:], ps2[:], bf2_b[:], op=ALU.add)
                nmx = work.tile([128, 1], _f32, tag="nmx")
                nc.vector.tensor_reduce(nmx[:], lg[:], axis=mybir.AxisListType.X,
                                        op=ALU.max, negate=True)
                ex = work.tile([128, D_OUT], _f32, tag="ex")
                sm = work.tile([128, 1], _f32, tag="sm")
                nc.scalar.activation(ex[:], lg[:], ACT.Exp, bias=nmx[:, :1], scale=1.0,
                                     accum_out=sm[:, :1])
                rin = work.tile([128, 1], _f32, tag="rin")
                nc.vector.reciprocal(rin[:], sm[:])
                ot = work.tile([128, D_OUT], _f32, tag="ot")
                nc.vector.tensor_scalar_mul(ot[:], ex[:], rin[:, :1])
                nc.sync.dma_start(out_d.ap()[m * 128:(m + 1) * 128, :], ot[:])

    nc.compile()
    return nc


def _run(inputs, trace=False):
    x = np.asarray(inputs["x"], dtype=np.float32)
    edge_index = np.asarray(inputs["edge_index"])
    deg = np.bincount(
        np.concatenate([edge_index[1], np.arange(N, dtype=edge_index.dtype)]),
        minlength=N,
    ).astype(np.float32)
    dinv = np.zeros(N, dtype=np.float32)
    nz = deg > 0
    dinv[nz] = (1.0 / np.sqrt(deg[nz])).astype(np.float32)

    per_core, group_sizes, chunk_base, TC = _prepare(edge_index, dinv)
    nc = _build(group_sizes, chunk_base, TC)

    if MODE == "bf16":
        import ml_dtypes
        mnp = ml_dtypes.bfloat16
    else:
        mnp = np.float32

    in_maps = []
    for c in range(NCORES):
        xp = np.zeros((RPAD, D), dtype=np.float32)
        xp[:RPC] = x[c * RPC:(c + 1) * RPC]
        dv = np.zeros(RPAD, dtype=np.float32)
        dv[:RPC] = dinv[c * RPC:(c + 1) * RPC]
        bb = {k: np.broadcast_to(np.asarray(inputs[k], np.float32), (128, D)).copy()
              for k in ("b1", "b2")}
        in_maps.append({
            "xT": np.ascontiguousarray(xp.T).astype(mnp),
            "dinv": dv,
            "W1": np.asarray(inputs["W1"], np.float32).astype(mnp),
            "W2": np.asarray(inputs["W2"], np.float32).astype(mnp),
            "Wf1": np.asarray(inputs["Wf1"], np.float32).astype(mnp),
            "Wf2": np.asarray(inputs["Wf2"], np.float32).astype(mnp),
            "b1": bb["b1"],
            "b2": bb["b2"],
            "bf1": np.asarray(inputs["bf1"], np.float32),
            "bf2": np.broadcast_to(np.asarray(inputs["bf2"], np.float32),
                                   (128, D_OUT)).copy(),
            "idx": per_core[c]["idx"],
            "wsl": per_core[c]["wsl"],
            "dvv": per_core[c]["dvv"],
        })

    res = run_bass_kernel_spmd(nc, in_maps, core_ids=list(range(NCORES)),
                               trace=trace)
    out = np.concatenate([res.results[c]["out"][:RPC] for c in range(NCORES)], axis=0)
    return out, res


def kernel(**inputs):
    out, _ = _run(inputs, trace=False)
    return out


# revision 8
# speedup vs baseline: 1.2356x; 1.2356x over previous
"""GCN (2x GCNConv + MLP head + softmax) on 8 TRN2 NeuronCores.

Strategy (graph/data parallel, per sharding hint):
  - Nodes sharded across 8 cores (2500 rows each, padded to 2560); weights
    replicated.
  - Per conv layer: each core computes h = z @ W for its rows, pre-scales
    rows by dinv (deg^-1/2), AllGathers the scaled table (node-major, in
    two halves overlapped with compute), then aggregates messages for
    edges partitioned by dst (windows of 128 dst slots) with dma_gather
    (row gather on 4 rotating SWDGE queues) + one-hot segment matmuls on
    the TensorEngine accumulating in PSUM. Segment matrices are built
    on-chip (DVE iota==slot * dinv); dinv[dst] is folded in; self-loops
    are ordinary edges. Window epilogue adds bias, relus, and
    PE-transposes into the feature-major layout the next matmul needs.
  - Head: two dense layers + row softmax, pipelined per 512-row block.

Host-side preprocessing is limited to graph-structure work (edge sort,
degree counts, window slots, gather-index layout) + sharding.
"""

import os
from contextlib import ExitStack

import numpy as np

import concourse.bacc as bacc
import concourse.mybir as mybir
import concourse.tile as tile
from concourse.bass_utils import run_bass_kernel_spmd
from concourse.masks import make_identity

# problem shapes (hardcoded per contract)
N = 20000
E = 320000
D = 512
D_OUT = 128
NCORES = 8
RPC = 2500          # real rows per core
RPAD = 2560         # padded rows per core (20 tiles of 128)
NPAD = RPAD * NCORES
MT = RPAD // 128    # m-tiles / dst windows per core (20)
HALF = RPAD // 2    # AG half-table rows per core
G = 6               # max chunks (of 128 edges) per dma_gather call
NQ = 4              # SWDGE queues for gather rotation

# config: "f32" (exact), "f32r" (fast fp32 matmul), "bf16" (half-traffic)
MODE = os.environ.get("GNN_MODE", "bf16")

_f32 = mybir.dt.float32
_f32r = mybir.dt.float32r
_bf16 = mybir.dt.bfloat16
_i16 = mybir.dt.int16
_i32 = mybir.dt.int32


def _table_id(node):
    """Row of node in the AllGather-ed table (two half-table AGs)."""
    c, r = node // RPC, node % RPC
    h = r // HALF
    return h * (HALF * NCORES) + c * HALF + (r - h * HALF)


def _prepare(edge_index, dinv):
    """Edge partitioning by dst; per-core gather indices + window slots.

    Returns (per_core list of dicts, group_sizes, chunk_base, TC).
    """
    src = np.concatenate([edge_index[0], np.arange(N, dtype=np.int64)])
    dst = np.concatenate([edge_index[1], np.arange(N, dtype=np.int64)])

    order = np.argsort(dst, kind="stable")
    srcs = _table_id(src[order]).astype(np.int64)
    dsts = dst[order]
    dinv_dst = dinv[dsts]

    counts = np.zeros((NCORES, MT), dtype=np.int64)
    core_bounds = np.searchsorted(dsts, np.arange(NCORES + 1) * RPC)
    for c in range(NCORES):
        lo, hi = core_bounds[c], core_bounds[c + 1]
        d = dsts[lo:hi] - c * RPC
        wb = np.searchsorted(d, np.arange(MT + 1) * 128)
        counts[c] = wb[1:] - wb[:-1]

    cpw = np.maximum(1, -(-counts.max(axis=0) // 128))  # chunks per window
    TC = int(cpw.sum())
    chunk_base = np.concatenate([[0], np.cumsum(cpw)]).astype(int)

    group_sizes = []
    for w in range(MT):
        n = int(cpw[w])
        gs = [G] * (n // G)
        if n % G:
            gs.append(n % G)
        group_sizes.append(gs)

    per_core = []
    for c in range(NCORES):
        gidx = np.zeros((TC, 128), dtype=np.int16)
        wsl = np.full((TC, 128), -1.0, dtype=np.float32)   # dst slot in window
        dvv = np.zeros((TC, 128), dtype=np.float32)        # dinv[dst]
        lo, hi = core_bounds[c], core_bounds[c + 1]
        d = dsts[lo:hi] - c * RPC
        s_ids = srcs[lo:hi]
        dv = dinv_dst[lo:hi]
        wb = np.searchsorted(d, np.arange(MT + 1) * 128)
        for w in range(MT):
            a, b = wb[w], wb[w + 1]
            k = np.arange(b - a)
            tg = chunk_base[w] + (k // 128)
            row = k % 128
            gidx[tg, row] = s_ids[a:b]
            wsl[tg, row] = (d[a:b] - w * 128).astype(np.float32)
            dvv[tg, row] = dv[a:b]
        # wrapped int16 index layout, one block per gather call
        cols = []
        for w in range(MT):
            t0 = chunk_base[w]
            for gsz in group_sizes[w]:
                L = gidx[t0:t0 + gsz].reshape(-1)
                cols.append(np.tile(L.reshape(-1, 16).T, (8, 1)))
                t0 += gsz
        idx_np = np.ascontiguousarray(np.concatenate(cols, axis=1))
        per_core.append({
            "idx": idx_np,
            "wsl": np.ascontiguousarray(wsl.T),   # [128, TC]
            "dvv": np.ascontiguousarray(dvv.T),   # [128, TC]
        })
    return per_core, group_sizes, chunk_base, TC


def _build(group_sizes, chunk_base, TC):
    # mdt: matmul-operand dtype; tdt: gathered-table dtype; trdt: transpose dtype
    mdt = {"f32": _f32, "f32r": _f32r, "bf16": _bf16}[MODE]
    tdt = _bf16 if MODE == "bf16" else _f32
    trdt = _bf16 if MODE == "bf16" else _f32

    nc = bacc.Bacc("TRN2", target_bir_lowering=False, debug=False,
                   num_devices=NCORES, num_swdge_queues=NQ)
    xT_d = nc.dram_tensor("xT", [D, RPAD], mdt, kind="ExternalInput")
    dinv_d = nc.dram_tensor("dinv", [RPAD], _f32, kind="ExternalInput")
    W_d = {k: nc.dram_tensor(k, [D, D], mdt, kind="ExternalInput")
           for k in ("W1", "W2", "Wf1")}
    Wf2_d = nc.dram_tensor("Wf2", [D, D_OUT], mdt, kind="ExternalInput")
    bb_d = {k: nc.dram_tensor(k, [128, D], _f32, kind="ExternalInput")
            for k in ("b1", "b2")}
    bf1_d = nc.dram_tensor("bf1", [D], _f32, kind="ExternalInput")
    bf2_d = nc.dram_tensor("bf2", [128, D_OUT], _f32, kind="ExternalInput")
    idx_d = nc.dram_tensor("idx", [128, TC * 8], _i16, kind="ExternalInput")
    wsl_d = nc.dram_tensor("wsl", [128, TC], _f32, kind="ExternalInput")
    out_d = nc.dram_tensor("out", [RPAD, D_OUT], _f32, kind="ExternalOutput")

    cc_in = [nc.dram_tensor(f"cc_in{i}", [RPAD, D], tdt, kind="Internal")
             for i in (1, 2)]
    cc_out = [nc.dram_tensor(f"cc_out{i}", [NPAD, D], tdt, kind="Internal",
                             addr_space="Shared") for i in (1, 2)]

    RG = [list(range(NCORES))]
    ACT = mybir.ActivationFunctionType
    ALU = mybir.AluOpType

    with tile.TileContext(nc) as tc, ExitStack() as ctx:
        const = ctx.enter_context(tc.tile_pool(name="const", bufs=1))
        actT = ctx.enter_context(tc.tile_pool(name="actT", bufs=2))
        work = ctx.enter_context(tc.tile_pool(name="work", bufs=2))
        msgp = ctx.enter_context(tc.tile_pool(name="msgp", bufs=8))
        sp = ctx.enter_context(tc.tile_pool(name="sp", bufs=5))
        psA = ctx.enter_context(tc.tile_pool(name="psA", bufs=2, space="PSUM"))
        psC = ctx.enter_context(tc.tile_pool(name="psC", bufs=2, space="PSUM"))
        psT = ctx.enter_context(tc.tile_pool(name="psT", bufs=2, space="PSUM"))

        # ---- constants ----
        w_t = {}
        for k in ("W1", "W2", "Wf1"):
            w_t[k] = const.tile([128, 4, D], mdt, name=f"wt_{k}")
            nc.sync.dma_start(w_t[k][:], W_d[k].ap().rearrange("(k p) n -> p k n", p=128))
        wf2_t = const.tile([128, 4, D_OUT], mdt)
        nc.sync.dma_start(wf2_t[:], Wf2_d.ap().rearrange("(k p) n -> p k n", p=128))
        b_b = {}
        for k in ("b1", "b2"):
            b_b[k] = const.tile([128, D], _f32, name=f"bb_{k}")
            nc.sync.dma_start(b_b[k][:], bb_d[k].ap())
        bf1_t = const.tile([128, 4], _f32)
        nc.sync.dma_start(bf1_t[:], bf1_d.ap().rearrange("(a p) -> p a", p=128))
        bf2_b = const.tile([128, D_OUT], _f32)
        nc.sync.dma_start(bf2_b[:], bf2_d.ap())
        dinv_t = const.tile([128, MT], _f32)
        nc.sync.dma_start(dinv_t[:], dinv_d.ap().rearrange("(a p) -> p a", p=128))
        ident = const.tile([128, 128], trdt)
        make_identity(nc, ident[:])
        idx_t = const.tile([128, TC * 8], _i16)
        nc.sync.dma_start(idx_t[:], idx_d.ap())
        wsl_t = const.tile([128, TC], _f32)
        nc.sync.dma_start(wsl_t[:], wsl_d.ap())
        iota_i = const.tile([128, G, 128], _i32)
        nc.gpsimd.iota(iota_i[:], pattern=[[0, G], [1, 128]], base=0, channel_multiplier=0)
        iota_f = const.tile([128, G, 128], _f32)
        nc.vector.tensor_copy(iota_f[:], iota_i[:])

        qn = [0]

        def phase_a(srcT, wt, cc, ms):
            # cc[m] = dinv * (z @ W) for this core's rows, m-tiles in ms
            for m in ms:
                ps = psA.tile([128, D], _f32, tag="psA")
                for k in range(4):
                    nc.tensor.matmul(ps[:], lhsT=srcT[:, k, m * 128:(m + 1) * 128],
                                     rhs=wt[:, k, :], start=(k == 0), stop=(k == 3))
                hs = work.tile([128, D], tdt, tag="hs")
                nc.scalar.activation(hs[:], ps[:], ACT.Copy, scale=dinv_t[:, m:m + 1])
                nc.sync.dma_start(cc.ap()[m * 128:(m + 1) * 128, :], hs[:])

        def allgather(i, h):
            nc.gpsimd.collective_compute(
                "AllGather", mybir.AluOpType.bypass,
                ins=[cc_in[i].ap()[h * HALF:(h + 1) * HALF]],
                outs=[cc_out[i].ap()[h * HALF * NCORES:(h + 1) * HALF * NCORES]],
                replica_groups=RG)

        def phase_c(cc, zT_next, bias_b, ws):
            # zT_next windows ws = relu(S^T-aggregated messages + b), transposed
            for w in ws:
                t0 = int(chunk_base[w])
                col0 = t0 * 8
                nchunks = sum(group_sizes[w])
                ps = psC.tile([128, D], _f32, tag="psC")
                done = 0
                for gsz in group_sizes[w]:
                    nidx = gsz * 128
                    msg = msgp.tile([128, G, D], mdt, tag="msg")
                    nc.gpsimd.dma_gather(msg[:, :gsz, :], cc.ap().bitcast(mdt),
                                         idx_t[:, col0:col0 + gsz * 8],
                                         nidx, nidx, D, queue_num=qn[0] % NQ)
                    qn[0] += 1
                    s_t = sp.tile([128, G, 128], mdt, tag="S")
                    nc.vector.tensor_tensor(
                        s_t[:, :gsz, :], iota_f[:, :gsz, :],
                        wsl_t[:, t0:t0 + gsz].to_broadcast([128, gsz, 128]),
                        op=ALU.is_equal)
                    for t in range(gsz):
                        nc.tensor.matmul(ps[:], lhsT=s_t[:, t, :], rhs=msg[:, t, :],
                                         start=(done == 0), stop=(done == nchunks - 1))
                        done += 1
                    t0 += gsz
                    col0 += gsz * 8
                zsc = work.tile([128, D], _f32, tag="zsc")
                nc.vector.tensor_scalar(zsc[:], ps[:], dinv_t[:, w:w + 1], None,
                                        op0=ALU.mult)
                zsum = work.tile([128, D], _f32, tag="zsum")
                nc.vector.tensor_tensor(zsum[:], zsc[:], bias_b[:], op=ALU.add)
                zrel = work.tile([128, D], trdt, tag="zrel")
                nc.scalar.activation(zrel[:], zsum[:], ACT.Relu)
                for q in range(4):
                    pt = psT.tile([128, 128], trdt, tag="psT")
                    nc.tensor.transpose(pt[:], zrel[:, q * 128:(q + 1) * 128], ident[:])
                    nc.vector.tensor_copy(zT_next[:, q, w * 128:(w + 1) * 128], pt[:])

        H2 = MT // 2

        # ---- layer 1 ----
        xT_t = actT.tile([128, 4, RPAD], mdt, tag="zT")
        nc.sync.dma_start(xT_t[:], xT_d.ap().rearrange("(k p) m -> p k m", p=128))
        phase_a(xT_t, w_t["W1"], cc_in[0], range(H2))
        allgather(0, 0)
        phase_a(xT_t, w_t["W1"], cc_in[0], range(H2, MT))
        allgather(0, 1)

        # ---- layer 1 aggregation interleaved with layer 2 phase A + AG2 ----
        z1T = actT.tile([128, 4, RPAD], mdt, tag="zT")
        phase_c(cc_out[0], z1T, b_b["b1"], range(H2))
        phase_a(z1T, w_t["W2"], cc_in[1], range(H2))
        allgather(1, 0)
        phase_c(cc_out[0], z1T, b_b["b1"], range(H2, MT))
        phase_a(z1T, w_t["W2"], cc_in[1], range(H2, MT))
        allgather(1, 1)

        # ---- layer 2 aggregation + head, pipelined per 512-row block ----
        z2T = actT.tile([128, 4, RPAD], mdt, tag="zT")
        z3T = actT.tile([128, 4, RPAD], mdt, tag="zT3")
        for mb in range(RPAD // 512):
            phase_c(cc_out[1], z2T, b_b["b2"], range(mb * 4, (mb + 1) * 4))
            for q in range(4):
                ps = psA.tile([128, D], _f32, tag="psA")
                for k in range(4):
                    nc.tensor.matmul(ps[:], lhsT=w_t["Wf1"][:, k, q * 128:(q + 1) * 128],
                                     rhs=z2T[:, k, mb * 512:(mb + 1) * 512],
                                     start=(k == 0), stop=(k == 3))
                nc.scalar.activation(z3T[:, q, mb * 512:(mb + 1) * 512], ps[:],
                                     ACT.Relu, bias=bf1_t[:, q:q + 1])
            for m in range(mb * 4, (mb + 1) * 4):
                ps2 = psT.tile([128, D_OUT], _f32, tag="psT")
                for k in range(4):
                    nc.tensor.matmul(ps2[:], lhsT=z3T[:, k, m * 128:(m + 1) * 128],
                                     rhs=wf2_t[:, k, :], start=(k == 0), stop=(k == 3))
                lg = work.tile([128, D_OUT], _f32, tag="lg")
                nc.vector.tensor_tensor(lg[:], ps2[:], bf2_b[:], op=ALU.add)
                nmx = work.tile([128, 1], _f32, tag="nmx")
                nc.vector.tensor_reduce(nmx[:], lg[:], axis=mybir.AxisListType.X,
                                        op=ALU.max, negate=True)
                ex = work.tile([128, D_OUT], _f32, tag="ex")
                sm = work.tile([128, 1], _f32, tag="sm")
                nc.scalar.activation(ex[:], lg[:], ACT.Exp, bias=nmx[:, :1], scale=1.0,
                                     accum_out=sm[:, :1])
                rin = work.tile([128, 1], _f32, tag="rin")
                nc.vector.reciprocal(rin[:], sm[:])
                ot = work.tile([128, D_OUT], _f32, tag="ot")
                nc.vector.tensor_scalar_mul(ot[:], ex[:], rin[:, :1])
                nc.sync.dma_start(out_d.ap()[m * 128:(m + 1) * 128, :], ot[:])

    nc.compile()
    return nc


def _run(inputs, trace=False):
    x = np.asarray(inputs["x"], dtype=np.float32)
    edge_index = np.asarray(inputs["edge_index"])
    deg = np.bincount(
        np.concatenate([edge_index[1], np.arange(N, dtype=edge_index.dtype)]),
        minlength=N,
    ).astype(np.float32)
    dinv = np.zeros(N, dtype=np.float32)
    nz = deg > 0
    dinv[nz] = (1.0 / np.sqrt(deg[nz])).astype(np.float32)

    per_core, group_sizes, chunk_base, TC = _prepare(edge_index, dinv)
    nc = _build(group_sizes, chunk_base, TC)

    if MODE == "bf16":
        import ml_dtypes
        mnp = ml_dtypes.bfloat16
    else:
        mnp = np.float32

    in_maps = []
    for c in range(NCORES):
        xp = np.zeros((RPAD, D), dtype=np.float32)
        xp[:RPC] = x[c * RPC:(c + 1) * RPC]
        dv = np.zeros(RPAD, dtype=np.float32)
        dv[:RPC] = dinv[c * RPC:(c + 1) * RPC]
        bb = {k: np.broadcast_to(np.asarray(inputs[k], np.float32), (128, D)).copy()
              for k in ("b1", "b2")}
        in_maps.append({
            "xT": np.ascontiguousarray(xp.T).astype(mnp),
            "dinv": dv,
            "W1": np.asarray(inputs["W1"], np.float32).astype(mnp),
            "W2": np.asarray(inputs["W2"], np.float32).astype(mnp),
            "Wf1": np.asarray(inputs["Wf1"], np.float32).astype(mnp),
            "Wf2": np.asarray(inputs["Wf2"], np.float32).astype(mnp),
            "b1": bb["b1"],
            "b2": bb["b2"],
            "bf1": np.asarray(inputs["bf1"], np.float32),
            "bf2": np.broadcast_to(np.asarray(inputs["bf2"], np.float32),
                                   (128, D_OUT)).copy(),
            "idx": per_core[c]["idx"],
            "wsl": per_core[c]["wsl"],
        })

    res = run_bass_kernel_spmd(nc, in_maps, core_ids=list(range(NCORES)),
                               trace=trace)
    out = np.concatenate([res.results[c]["out"][:RPC] for c in range(NCORES)], axis=0)
    return out, res


def kernel(**inputs):
    out, _ = _run(inputs, trace=False)
    return out


# revision 9
# speedup vs baseline: 1.2486x; 1.0106x over previous
"""GCN (2x GCNConv + MLP head + softmax) on 8 TRN2 NeuronCores.

Strategy (graph/data parallel, per sharding hint):
  - Nodes sharded across 8 cores (2500 rows each, padded to 2560); weights
    replicated.
  - Per conv layer: each core computes h = z @ W for its rows, pre-scales
    rows by dinv (deg^-1/2), AllGathers the scaled table (node-major, in
    two halves overlapped with compute), then aggregates messages for
    edges partitioned by dst (windows of 128 dst slots) with dma_gather
    (row gather on 4 rotating SWDGE queues) + one-hot segment matmuls on
    the TensorEngine accumulating in PSUM. Segment matrices are built
    on-chip (DVE iota==slot * dinv); dinv[dst] is folded in; self-loops
    are ordinary edges. Window epilogue adds bias, relus, and
    PE-transposes into the feature-major layout the next matmul needs.
  - Head: two dense layers + row softmax, pipelined per 512-row block.

Host-side preprocessing is limited to graph-structure work (edge sort,
degree counts, window slots, gather-index layout) + sharding.
"""

import os
from contextlib import ExitStack

import numpy as np

import concourse.bacc as bacc
import concourse.mybir as mybir
import concourse.tile as tile
from concourse.bass_utils import run_bass_kernel_spmd
from concourse.masks import make_identity

# problem shapes (hardcoded per contract)
N = 20000
E = 320000
D = 512
D_OUT = 128
NCORES = 8
RPC = 2500          # real rows per core
RPAD = 2560         # padded rows per core (20 tiles of 128)
NPAD = RPAD * NCORES
MT = RPAD // 128    # m-tiles / dst windows per core (20)
HALF = RPAD // 2    # AG half-table rows per core
G = 6               # max chunks (of 128 edges) per dma_gather call
NQ = 4              # SWDGE queues for gather rotation

# config: "f32" (exact), "f32r" (fast fp32 matmul), "bf16" (half-traffic)
MODE = os.environ.get("GNN_MODE", "bf16")

_f32 = mybir.dt.float32
_f32r = mybir.dt.float32r
_bf16 = mybir.dt.bfloat16
_i16 = mybir.dt.int16
_i32 = mybir.dt.int32


def _table_id(node):
    """Row of node in the AllGather-ed table (two half-table AGs)."""
    c, r = node // RPC, node % RPC
    h = r // HALF
    return h * (HALF * NCORES) + c * HALF + (r - h * HALF)


def _prepare(edge_index, dinv):
    """Edge partitioning by dst; per-core gather indices + window slots.

    Returns (per_core list of dicts, group_sizes, chunk_base, TC).
    """
    src = np.concatenate([edge_index[0], np.arange(N, dtype=np.int64)])
    dst = np.concatenate([edge_index[1], np.arange(N, dtype=np.int64)])

    order = np.argsort(dst, kind="stable")
    srcs = _table_id(src[order]).astype(np.int64)
    dsts = dst[order]
    dinv_dst = dinv[dsts]

    counts = np.zeros((NCORES, MT), dtype=np.int64)
    core_bounds = np.searchsorted(dsts, np.arange(NCORES + 1) * RPC)
    for c in range(NCORES):
        lo, hi = core_bounds[c], core_bounds[c + 1]
        d = dsts[lo:hi] - c * RPC
        wb = np.searchsorted(d, np.arange(MT + 1) * 128)
        counts[c] = wb[1:] - wb[:-1]

    cpw = np.maximum(1, -(-counts.max(axis=0) // 128))  # chunks per window
    TC = int(cpw.sum())
    chunk_base = np.concatenate([[0], np.cumsum(cpw)]).astype(int)

    group_sizes = []
    for w in range(MT):
        n = int(cpw[w])
        gs = [G] * (n // G)
        if n % G:
            gs.append(n % G)
        group_sizes.append(gs)

    per_core = []
    for c in range(NCORES):
        gidx = np.zeros((TC, 128), dtype=np.int16)
        wsl = np.full((TC, 128), -1.0, dtype=np.float32)   # dst slot in window
        dvv = np.zeros((TC, 128), dtype=np.float32)        # dinv[dst]
        lo, hi = core_bounds[c], core_bounds[c + 1]
        d = dsts[lo:hi] - c * RPC
        s_ids = srcs[lo:hi]
        dv = dinv_dst[lo:hi]
        wb = np.searchsorted(d, np.arange(MT + 1) * 128)
        for w in range(MT):
            a, b = wb[w], wb[w + 1]
            k = np.arange(b - a)
            tg = chunk_base[w] + (k // 128)
            row = k % 128
            gidx[tg, row] = s_ids[a:b]
            wsl[tg, row] = (d[a:b] - w * 128).astype(np.float32)
            dvv[tg, row] = dv[a:b]
        # wrapped int16 index layout, one block per gather call
        cols = []
        for w in range(MT):
            t0 = chunk_base[w]
            for gsz in group_sizes[w]:
                L = gidx[t0:t0 + gsz].reshape(-1)
                cols.append(np.tile(L.reshape(-1, 16).T, (8, 1)))
                t0 += gsz
        idx_np = np.ascontiguousarray(np.concatenate(cols, axis=1))
        per_core.append({
            "idx": idx_np,
            "wsl": np.ascontiguousarray(wsl.T),   # [128, TC]
            "dvv": np.ascontiguousarray(dvv.T),   # [128, TC]
        })
    return per_core, group_sizes, chunk_base, TC


def _build(group_sizes, chunk_base, TC):
    # mdt: matmul-operand dtype; tdt: gathered-table dtype; trdt: transpose dtype
    mdt = {"f32": _f32, "f32r": _f32r, "bf16": _bf16}[MODE]
    tdt = _bf16 if MODE == "bf16" else _f32
    trdt = _bf16 if MODE == "bf16" else _f32

    nc = bacc.Bacc("TRN2", target_bir_lowering=False, debug=False,
                   num_devices=NCORES, num_swdge_queues=NQ)
    xT_d = nc.dram_tensor("xT", [D, RPAD], mdt, kind="ExternalInput")
    dinv_d = nc.dram_tensor("dinv", [RPAD], _f32, kind="ExternalInput")
    W_d = {k: nc.dram_tensor(k, [D, D], mdt, kind="ExternalInput")
           for k in ("W1", "W2", "Wf1")}
    Wf2_d = nc.dram_tensor("Wf2", [D, D_OUT], mdt, kind="ExternalInput")
    bb_d = {k: nc.dram_tensor(k, [128, D], _f32, kind="ExternalInput")
            for k in ("b1", "b2")}
    bf1_d = nc.dram_tensor("bf1", [D], _f32, kind="ExternalInput")
    bf2_d = nc.dram_tensor("bf2", [128, D_OUT], _f32, kind="ExternalInput")
    idx_d = nc.dram_tensor("idx", [128, TC * 8], _i16, kind="ExternalInput")
    wsl_d = nc.dram_tensor("wsl", [128, TC], _f32, kind="ExternalInput")
    out_d = nc.dram_tensor("out", [RPAD, D_OUT], _f32, kind="ExternalOutput")

    cc_in = [nc.dram_tensor(f"cc_in{i}", [RPAD, D], tdt, kind="Internal")
             for i in (1, 2)]
    cc_out = [nc.dram_tensor(f"cc_out{i}", [NPAD, D], tdt, kind="Internal",
                             addr_space="Shared") for i in (1, 2)]

    RG = [list(range(NCORES))]
    ACT = mybir.ActivationFunctionType
    ALU = mybir.AluOpType

    with tile.TileContext(nc) as tc, ExitStack() as ctx:
        const = ctx.enter_context(tc.tile_pool(name="const", bufs=1))
        actT = ctx.enter_context(tc.tile_pool(name="actT", bufs=2))
        work = ctx.enter_context(tc.tile_pool(name="work", bufs=3))
        msgp = ctx.enter_context(tc.tile_pool(name="msgp", bufs=8))
        sp = ctx.enter_context(tc.tile_pool(name="sp", bufs=5))
        psA = ctx.enter_context(tc.tile_pool(name="psA", bufs=2, space="PSUM"))
        psC = ctx.enter_context(tc.tile_pool(name="psC", bufs=3, space="PSUM"))
        psT = ctx.enter_context(tc.tile_pool(name="psT", bufs=2, space="PSUM"))

        # ---- constants ----
        w_t = {}
        for k in ("W1", "W2", "Wf1"):
            w_t[k] = const.tile([128, 4, D], mdt, name=f"wt_{k}")
            nc.sync.dma_start(w_t[k][:], W_d[k].ap().rearrange("(k p) n -> p k n", p=128))
        wf2_t = const.tile([128, 4, D_OUT], mdt)
        nc.sync.dma_start(wf2_t[:], Wf2_d.ap().rearrange("(k p) n -> p k n", p=128))
        b_b = {}
        for k in ("b1", "b2"):
            b_b[k] = const.tile([128, D], _f32, name=f"bb_{k}")
            nc.sync.dma_start(b_b[k][:], bb_d[k].ap())
        bf1_t = const.tile([128, 4], _f32)
        nc.sync.dma_start(bf1_t[:], bf1_d.ap().rearrange("(a p) -> p a", p=128))
        bf2_b = const.tile([128, D_OUT], _f32)
        nc.sync.dma_start(bf2_b[:], bf2_d.ap())
        dinv_t = const.tile([128, MT], _f32)
        nc.sync.dma_start(dinv_t[:], dinv_d.ap().rearrange("(a p) -> p a", p=128))
        ident = const.tile([128, 128], trdt)
        make_identity(nc, ident[:])
        idx_t = const.tile([128, TC * 8], _i16)
        nc.sync.dma_start(idx_t[:], idx_d.ap())
        wsl_t = const.tile([128, TC], _f32)
        nc.sync.dma_start(wsl_t[:], wsl_d.ap())
        iota_i = const.tile([128, G, 128], _i32)
        nc.gpsimd.iota(iota_i[:], pattern=[[0, G], [1, 128]], base=0, channel_multiplier=0)
        iota_f = const.tile([128, G, 128], _f32)
        nc.vector.tensor_copy(iota_f[:], iota_i[:])

        qn = [0]

        def phase_a(srcT, wt, cc, ms):
            # cc[m] = dinv * (z @ W) for this core's rows, m-tiles in ms
            for m in ms:
                ps = psA.tile([128, D], _f32, tag="psA")
                for k in range(4):
                    nc.tensor.matmul(ps[:], lhsT=srcT[:, k, m * 128:(m + 1) * 128],
                                     rhs=wt[:, k, :], start=(k == 0), stop=(k == 3))
                hs = work.tile([128, D], tdt, tag="hs")
                nc.scalar.activation(hs[:], ps[:], ACT.Copy, scale=dinv_t[:, m:m + 1])
                nc.sync.dma_start(cc.ap()[m * 128:(m + 1) * 128, :], hs[:])

        def allgather(i, h):
            nc.gpsimd.collective_compute(
                "AllGather", mybir.AluOpType.bypass,
                ins=[cc_in[i].ap()[h * HALF:(h + 1) * HALF]],
                outs=[cc_out[i].ap()[h * HALF * NCORES:(h + 1) * HALF * NCORES]],
                replica_groups=RG)

        def phase_c(cc, zT_next, bias_b, ws):
            # zT_next windows ws = relu(S^T-aggregated messages + b), transposed
            for w in ws:
                t0 = int(chunk_base[w])
                col0 = t0 * 8
                nchunks = sum(group_sizes[w])
                ps = psC.tile([128, D], _f32, tag="psC")
                done = 0
                for gsz in group_sizes[w]:
                    nidx = gsz * 128
                    msg = msgp.tile([128, G, D], mdt, tag="msg")
                    nc.gpsimd.dma_gather(msg[:, :gsz, :], cc.ap().bitcast(mdt),
                                         idx_t[:, col0:col0 + gsz * 8],
                                         nidx, nidx, D, queue_num=qn[0] % NQ)
                    qn[0] += 1
                    s_t = sp.tile([128, G, 128], mdt, tag="S")
                    nc.vector.tensor_tensor(
                        s_t[:, :gsz, :], iota_f[:, :gsz, :],
                        wsl_t[:, t0:t0 + gsz].to_broadcast([128, gsz, 128]),
                        op=ALU.is_equal)
                    for t in range(gsz):
                        nc.tensor.matmul(ps[:], lhsT=s_t[:, t, :], rhs=msg[:, t, :],
                                         start=(done == 0), stop=(done == nchunks - 1))
                        done += 1
                    t0 += gsz
                    col0 += gsz * 8
                zsc = work.tile([128, D], _f32, tag="zsc")
                nc.vector.tensor_scalar(zsc[:], ps[:], dinv_t[:, w:w + 1], None,
                                        op0=ALU.mult)
                zsum = work.tile([128, D], _f32, tag="zsum")
                nc.vector.tensor_tensor(zsum[:], zsc[:], bias_b[:], op=ALU.add)
                zrel = work.tile([128, D], trdt, tag="zrel")
                nc.scalar.activation(zrel[:], zsum[:], ACT.Relu)
                for q in range(4):
                    pt = psT.tile([128, 128], trdt, tag="psT")
                    nc.tensor.transpose(pt[:], zrel[:, q * 128:(q + 1) * 128], ident[:])
                    nc.vector.tensor_copy(zT_next[:, q, w * 128:(w + 1) * 128], pt[:])

        H2 = MT // 2

        # ---- layer 1 ----
        xT_t = actT.tile([128, 4, RPAD], mdt, tag="zT")
        for m in range(MT):
            nc.sync.dma_start(
                xT_t[:, :, m * 128:(m + 1) * 128],
                xT_d.ap()[:, m * 128:(m + 1) * 128].rearrange("(k p) m -> p k m", p=128))
        phase_a(xT_t, w_t["W1"], cc_in[0], range(H2))
        allgather(0, 0)
        phase_a(xT_t, w_t["W1"], cc_in[0], range(H2, MT))
        allgather(0, 1)

        # ---- layer 1 aggregation interleaved with layer 2 phase A + AG2 ----
        z1T = actT.tile([128, 4, RPAD], mdt, tag="zT")
        phase_c(cc_out[0], z1T, b_b["b1"], range(H2))
        phase_a(z1T, w_t["W2"], cc_in[1], range(H2))
        allgather(1, 0)
        phase_c(cc_out[0], z1T, b_b["b1"], range(H2, MT))
        phase_a(z1T, w_t["W2"], cc_in[1], range(H2, MT))
        allgather(1, 1)

        # ---- layer 2 aggregation + head, pipelined per 512-row block ----
        z2T = actT.tile([128, 4, RPAD], mdt, tag="zT")
        z3T = actT.tile([128, 4, RPAD], mdt, tag="zT3")
        for mb in range(RPAD // 512):
            phase_c(cc_out[1], z2T, b_b["b2"], range(mb * 4, (mb + 1) * 4))
            for q in range(4):
                ps = psA.tile([128, D], _f32, tag="psA")
                for k in range(4):
                    nc.tensor.matmul(ps[:], lhsT=w_t["Wf1"][:, k, q * 128:(q + 1) * 128],
                                     rhs=z2T[:, k, mb * 512:(mb + 1) * 512],
                                     start=(k == 0), stop=(k == 3))
                nc.scalar.activation(z3T[:, q, mb * 512:(mb + 1) * 512], ps[:],
                                     ACT.Relu, bias=bf1_t[:, q:q + 1])
            for m in range(mb * 4, (mb + 1) * 4):
                ps2 = psT.tile([128, D_OUT], _f32, tag="psT")
                for k in range(4):
                    nc.tensor.matmul(ps2[:], lhsT=z3T[:, k, m * 128:(m + 1) * 128],
                                     rhs=wf2_t[:, k, :], start=(k == 0), stop=(k == 3))
                lg = work.tile([128, D_OUT], _f32, tag="lg")
                nc.vector.tensor_tensor(lg[:], ps2[:], bf2_b[:], op=ALU.add)
                nmx = work.tile([128, 1], _f32, tag="nmx")
                nc.vector.tensor_reduce(nmx[:], lg[:], axis=mybir.AxisListType.X,
                                        op=ALU.max, negate=True)
                ex = work.tile([128, D_OUT], _f32, tag="ex")
                sm = work.tile([128, 1], _f32, tag="sm")
                nc.scalar.activation(ex[:], lg[:], ACT.Exp, bias=nmx[:, :1], scale=1.0,
                                     accum_out=sm[:, :1])
                rin = work.tile([128, 1], _f32, tag="rin")
                nc.vector.reciprocal(rin[:], sm[:])
                ot = work.tile([128, D_OUT], _f32, tag="ot")
                nc.vector.tensor_scalar_mul(ot[:], ex[:], rin[:, :1])
                nc.sync.dma_start(out_d.ap()[m * 128:(m + 1) * 128, :], ot[:])

    nc.compile()
    return nc


def _run(inputs, trace=False):
    x = np.asarray(inputs["x"], dtype=np.float32)
    edge_index = np.asarray(inputs["edge_index"])
    deg = np.bincount(
        np.concatenate([edge_index[1], np.arange(N, dtype=edge_index.dtype)]),
        minlength=N,
    ).astype(np.float32)
    dinv = np.zeros(N, dtype=np.float32)
    nz = deg > 0
    dinv[nz] = (1.0 / np.sqrt(deg[nz])).astype(np.float32)

    per_core, group_sizes, chunk_base, TC = _prepare(edge_index, dinv)
    nc = _build(group_sizes, chunk_base, TC)

    if MODE == "bf16":
        import ml_dtypes
        mnp = ml_dtypes.bfloat16
    else:
        mnp = np.float32

    in_maps = []
    for c in range(NCORES):
        xp = np.zeros((RPAD, D), dtype=np.float32)
        xp[:RPC] = x[c * RPC:(c + 1) * RPC]
        dv = np.zeros(RPAD, dtype=np.float32)
        dv[:RPC] = dinv[c * RPC:(c + 1) * RPC]
        bb = {k: np.broadcast_to(np.asarray(inputs[k], np.float32), (128, D)).copy()
              for k in ("b1", "b2")}
        in_maps.append({
            "xT": np.ascontiguousarray(xp.T).astype(mnp),
            "dinv": dv,
            "W1": np.asarray(inputs["W1"], np.float32).astype(mnp),
            "W2": np.asarray(inputs["W2"], np.float32).astype(mnp),
            "Wf1": np.asarray(inputs["Wf1"], np.float32).astype(mnp),
            "Wf2": np.asarray(inputs["Wf2"], np.float32).astype(mnp),
            "b1": bb["b1"],
            "b2": bb["b2"],
            "bf1": np.asarray(inputs["bf1"], np.float32),
            "bf2": np.broadcast_to(np.asarray(inputs["bf2"], np.float32),
                                   (128, D_OUT)).copy(),
            "idx": per_core[c]["idx"],
            "wsl": per_core[c]["wsl"],
        })

    res = run_bass_kernel_spmd(nc, in_maps, core_ids=list(range(NCORES)),
                               trace=trace)
    out = np.concatenate([res.results[c]["out"][:RPC] for c in range(NCORES)], axis=0)
    return out, res


def kernel(**inputs):
    out, _ = _run(inputs, trace=False)
    return out


# revision 13
# speedup vs baseline: 1.2668x; 1.0146x over previous
"""GCN (2x GCNConv + MLP head + softmax) on 8 TRN2 NeuronCores.

Strategy (graph/data parallel, per sharding hint):
  - Nodes sharded across 8 cores (2500 rows each, padded to 2560); weights
    replicated.
  - Per conv layer: each core computes h = z @ W for its rows, pre-scales
    rows by dinv (deg^-1/2), AllGathers the scaled table (node-major, in
    two halves overlapped with compute), then aggregates messages for
    edges partitioned by dst (windows of 128 dst slots) with dma_gather
    (row gather on 4 rotating SWDGE queues) + one-hot segment matmuls on
    the TensorEngine accumulating in PSUM. Segment matrices are built
    on-chip (DVE iota==slot * dinv); dinv[dst] is folded in; self-loops
    are ordinary edges. Window epilogue adds bias, relus, and
    PE-transposes into the feature-major layout the next matmul needs.
  - Head: two dense layers + row softmax, pipelined per 512-row block.

Host-side preprocessing is limited to graph-structure work (edge sort,
degree counts, window slots, gather-index layout) + sharding.
"""

import os
from contextlib import ExitStack

import numpy as np

import concourse.bacc as bacc
import concourse.mybir as mybir
import concourse.tile as tile
from concourse.bass_utils import run_bass_kernel_spmd
from concourse.masks import make_identity

# problem shapes (hardcoded per contract)
N = 20000
E = 320000
D = 512
D_OUT = 128
NCORES = 8
RPC = 2500          # real rows per core
RPAD = 2560         # padded rows per core (20 tiles of 128)
NPAD = RPAD * NCORES
MT = RPAD // 128    # m-tiles / dst windows per core (20)
HALF = RPAD // 2    # AG half-table rows per core
G = 6               # max chunks (of 128 edges) per dma_gather call
NQ = 4              # SWDGE queues for gather rotation

# config: "f32" (exact), "f32r" (fast fp32 matmul), "bf16" (half-traffic)
MODE = os.environ.get("GNN_MODE", "bf16")

_f32 = mybir.dt.float32
_f32r = mybir.dt.float32r
_bf16 = mybir.dt.bfloat16
_i16 = mybir.dt.int16
_i32 = mybir.dt.int32


def _table_id(node):
    """Row of node in the AllGather-ed table (two half-table AGs)."""
    c, r = node // RPC, node % RPC
    h = r // HALF
    return h * (HALF * NCORES) + c * HALF + (r - h * HALF)


def _prepare(edge_index, dinv):
    """Edge partitioning by dst; per-core gather indices + window slots.

    Returns (per_core list of dicts, group_sizes, chunk_base, TC).
    """
    src = np.concatenate([edge_index[0], np.arange(N, dtype=np.int64)])
    dst = np.concatenate([edge_index[1], np.arange(N, dtype=np.int64)])

    order = np.argsort(dst, kind="stable")
    srcs = _table_id(src[order]).astype(np.int64)
    dsts = dst[order]

    HB = HALF * NCORES  # table rows in half 0

    counts = np.zeros((NCORES, MT, 2), dtype=np.int64)
    core_bounds = np.searchsorted(dsts, np.arange(NCORES + 1) * RPC)
    for c in range(NCORES):
        lo, hi = core_bounds[c], core_bounds[c + 1]
        d = dsts[lo:hi] - c * RPC
        hvec = (srcs[lo:hi] >= HB).astype(np.int64)
        wb = np.searchsorted(d, np.arange(MT + 1) * 128)
        for w in range(MT):
            a, b = wb[w], wb[w + 1]
            n1 = int(hvec[a:b].sum())
            counts[c, w, 0] = (b - a) - n1
            counts[c, w, 1] = n1

    cpw = np.maximum(1, -(-counts.max(axis=0) // 128))  # [MT, 2] chunks
    TC = int(cpw.sum())
    cb = np.concatenate([[0], np.cumsum(cpw.reshape(-1))]).astype(int)
    chunk_base = cb[:-1].reshape(MT, 2)

    group_sizes = []
    for w in range(MT):
        gw = []
        for h in (0, 1):
            n = int(cpw[w, h])
            gs = [G] * (n // G)
            if n % G:
                gs.append(n % G)
            gw.append(gs)
        group_sizes.append(gw)

    per_core = []
    for c in range(NCORES):
        gidx = np.zeros((TC, 128), dtype=np.int16)
        wsl = np.full((TC, 128), -1.0, dtype=np.float32)   # dst slot in window
        lo, hi = core_bounds[c], core_bounds[c + 1]
        d = dsts[lo:hi] - c * RPC
        s_ids = srcs[lo:hi]
        wb = np.searchsorted(d, np.arange(MT + 1) * 128)
        for w in range(MT):
            a, b = wb[w], wb[w + 1]
            hv = s_ids[a:b] >= HB
            for h in (0, 1):
                sel = hv if h else ~hv
                sid = (s_ids[a:b][sel] - h * HB).astype(np.int16)
                slot = (d[a:b][sel] - w * 128).astype(np.float32)
                k = np.arange(sid.size)
                tg = chunk_base[w, h] + (k // 128)
                row = k % 128
                gidx[tg, row] = sid
                wsl[tg, row] = slot
        # wrapped int16 index layout, one block per gather call
        cols = []
        for w in range(MT):
            for h in (0, 1):
                t0 = int(chunk_base[w, h])
                for gsz in group_sizes[w][h]:
                    L = gidx[t0:t0 + gsz].reshape(-1)
                    cols.append(np.tile(L.reshape(-1, 16).T, (8, 1)))
                    t0 += gsz
        idx_np = np.ascontiguousarray(np.concatenate(cols, axis=1))
        per_core.append({
            "idx": idx_np,
            "wsl": np.ascontiguousarray(wsl.T),   # [128, TC]
        })
    return per_core, group_sizes, chunk_base, TC


def _build(group_sizes, chunk_base, TC):
    # mdt: matmul-operand dtype; tdt: gathered-table dtype; trdt: transpose dtype
    mdt = {"f32": _f32, "f32r": _f32r, "bf16": _bf16}[MODE]
    tdt = _bf16 if MODE == "bf16" else _f32
    trdt = _bf16 if MODE == "bf16" else _f32

    nc = bacc.Bacc("TRN2", target_bir_lowering=False, debug=False,
                   num_devices=NCORES, num_swdge_queues=NQ)
    xT_d = nc.dram_tensor("xT", [D, RPAD], mdt, kind="ExternalInput")
    dinv_d = nc.dram_tensor("dinv", [RPAD], _f32, kind="ExternalInput")
    W_d = {k: nc.dram_tensor(k, [D, D], mdt, kind="ExternalInput")
           for k in ("W1", "W2", "Wf1")}
    Wf2_d = nc.dram_tensor("Wf2", [D, D_OUT], mdt, kind="ExternalInput")
    bb_d = {k: nc.dram_tensor(k, [128, D], _f32, kind="ExternalInput")
            for k in ("b1", "b2")}
    bf1_d = nc.dram_tensor("bf1", [D], _f32, kind="ExternalInput")
    bf2_d = nc.dram_tensor("bf2", [128, D_OUT], _f32, kind="ExternalInput")
    idx_d = nc.dram_tensor("idx", [128, TC * 8], _i16, kind="ExternalInput")
    wsl_d = nc.dram_tensor("wsl", [128, TC], _f32, kind="ExternalInput")
    out_d = nc.dram_tensor("out", [RPAD, D_OUT], _f32, kind="ExternalOutput")

    cc_in = [nc.dram_tensor(f"cc_in{i}", [RPAD, D], tdt, kind="Internal")
             for i in (1, 2)]
    cc_out = [nc.dram_tensor(f"cc_out{i}", [NPAD, D], tdt, kind="Internal",
                             addr_space="Shared") for i in (1, 2)]

    RG = [list(range(NCORES))]
    ACT = mybir.ActivationFunctionType
    ALU = mybir.AluOpType

    with tile.TileContext(nc) as tc, ExitStack() as ctx:
        const = ctx.enter_context(tc.tile_pool(name="const", bufs=1))
        actT = ctx.enter_context(tc.tile_pool(name="actT", bufs=2))
        work = ctx.enter_context(tc.tile_pool(name="work", bufs=3))
        msgp = ctx.enter_context(tc.tile_pool(name="msgp", bufs=8))
        sp = ctx.enter_context(tc.tile_pool(name="sp", bufs=5))
        psA = ctx.enter_context(tc.tile_pool(name="psA", bufs=2, space="PSUM"))
        psC = ctx.enter_context(tc.tile_pool(name="psC", bufs=4, space="PSUM"))
        psT = ctx.enter_context(tc.tile_pool(name="psT", bufs=2, space="PSUM"))

        # ---- constants ----
        w_t = {}
        for k in ("W1", "W2", "Wf1"):
            w_t[k] = const.tile([128, 4, D], mdt, name=f"wt_{k}")
            nc.sync.dma_start(w_t[k][:], W_d[k].ap().rearrange("(k p) n -> p k n", p=128))
        wf2_t = const.tile([128, 4, D_OUT], mdt)
        nc.sync.dma_start(wf2_t[:], Wf2_d.ap().rearrange("(k p) n -> p k n", p=128))
        b_b = {}
        for k in ("b1", "b2"):
            b_b[k] = const.tile([128, D], _f32, name=f"bb_{k}")
            nc.sync.dma_start(b_b[k][:], bb_d[k].ap())
        bf1_t = const.tile([128, 4], _f32)
        nc.sync.dma_start(bf1_t[:], bf1_d.ap().rearrange("(a p) -> p a", p=128))
        bf2_b = const.tile([128, D_OUT], _f32)
        nc.sync.dma_start(bf2_b[:], bf2_d.ap())
        dinv_t = const.tile([128, MT], _f32)
        nc.sync.dma_start(dinv_t[:], dinv_d.ap().rearrange("(a p) -> p a", p=128))
        ident = const.tile([128, 128], trdt)
        make_identity(nc, ident[:])
        idx_t = const.tile([128, TC * 8], _i16)
        nc.sync.dma_start(idx_t[:], idx_d.ap())
        wsl_t = const.tile([128, TC], _f32)
        nc.sync.dma_start(wsl_t[:], wsl_d.ap())
        iota_i = const.tile([128, G, 128], _i32)
        nc.gpsimd.iota(iota_i[:], pattern=[[0, G], [1, 128]], base=0, channel_multiplier=0)
        iota_f = const.tile([128, G, 128], _f32)
        nc.vector.tensor_copy(iota_f[:], iota_i[:])

        qn = [0]

        def phase_a(srcT, wt, cc, ms):
            # cc[m] = dinv * (z @ W) for this core's rows, m-tiles in ms
            for m in ms:
                ps = psA.tile([128, D], _f32, tag="psA")
                for k in range(4):
                    nc.tensor.matmul(ps[:], lhsT=srcT[:, k, m * 128:(m + 1) * 128],
                                     rhs=wt[:, k, :], start=(k == 0), stop=(k == 3))
                hs = work.tile([128, D], tdt, tag="hs")
                nc.scalar.activation(hs[:], ps[:], ACT.Copy, scale=dinv_t[:, m:m + 1])
                nc.sync.dma_start(cc.ap()[m * 128:(m + 1) * 128, :], hs[:])

        def allgather(i, h):
            nc.gpsimd.collective_compute(
                "AllGather", mybir.AluOpType.bypass,
                ins=[cc_in[i].ap()[h * HALF:(h + 1) * HALF]],
                outs=[cc_out[i].ap()[h * HALF * NCORES:(h + 1) * HALF * NCORES]],
                replica_groups=RG)

        def phase_c(cc, zT_next, bias_b, after_epilogue=None):
            """Pipelined window aggregation. Half-0 gathers only depend on the
            first AG half (sliced source AP); lookahead L windows run their
            half-0 stage early so gather work overlaps the second AG half."""
            LA = 3
            st = {}
            HB = HALF * NCORES

            def h_stage(w, h):
                if w not in st:
                    st[w] = [psC.tile([128, D], _f32, tag="psC", name=f"psC_{w}"), 0]
                ps, _ = st[w]
                total = sum(group_sizes[w][0]) + sum(group_sizes[w][1])
                t0 = int(chunk_base[w][h])
                col0 = t0 * 8
                src_ap = cc.ap()[h * HB:(h + 1) * HB].bitcast(mdt)
                for gsz in group_sizes[w][h]:
                    nidx = gsz * 128
                    msg = msgp.tile([128, G, D], mdt, tag="msg")
                    nc.gpsimd.dma_gather(msg[:, :gsz, :], src_ap,
                                         idx_t[:, col0:col0 + gsz * 8],
                                         nidx, nidx, D, queue_num=qn[0] % NQ)
                    qn[0] += 1
                    s_t = sp.tile([128, G, 128], mdt, tag="S")
                    nc.vector.tensor_tensor(
                        s_t[:, :gsz, :], iota_f[:, :gsz, :],
                        wsl_t[:, t0:t0 + gsz].to_broadcast([128, gsz, 128]),
                        op=ALU.is_equal)
                    for t in range(gsz):
                        done = st[w][1]
                        nc.tensor.matmul(ps[:], lhsT=s_t[:, t, :], rhs=msg[:, t, :],
                                         start=(done == 0), stop=(done == total - 1))
                        st[w][1] = done + 1
                    t0 += gsz
                    col0 += gsz * 8

            def epilogue(w):
                ps, _ = st.pop(w)
                zsc = work.tile([128, D], _f32, tag="zsc")
                nc.vector.tensor_scalar(zsc[:], ps[:], dinv_t[:, w:w + 1], None,
                                        op0=ALU.mult)
                zsum = work.tile([128, D], _f32, tag="zsum")
                nc.vector.tensor_tensor(zsum[:], zsc[:], bias_b[:], op=ALU.add)
                zrel = work.tile([128, D], trdt, tag="zrel")
                nc.scalar.activation(zrel[:], zsum[:], ACT.Relu)
                for q in range(4):
                    pt = psT.tile([128, 128], trdt, tag="psT")
                    nc.tensor.transpose(pt[:], zrel[:, q * 128:(q + 1) * 128], ident[:])
                    nc.scalar.copy(zT_next[:, q, w * 128:(w + 1) * 128], pt[:])

            for w in range(LA):
                h_stage(w, 0)
            for w in range(MT):
                h_stage(w, 1)
                epilogue(w)
                if w + LA < MT:
                    h_stage(w + LA, 0)
                if after_epilogue is not None:
                    after_epilogue(w)

        H2 = MT // 2

        # ---- layer 1 ----
        xT_t = actT.tile([128, 4, RPAD], mdt, tag="zT")
        for m in range(MT):
            nc.sync.dma_start(
                xT_t[:, :, m * 128:(m + 1) * 128],
                xT_d.ap()[:, m * 128:(m + 1) * 128].rearrange("(k p) m -> p k m", p=128))
        phase_a(xT_t, w_t["W1"], cc_in[0], range(H2))
        allgather(0, 0)
        phase_a(xT_t, w_t["W1"], cc_in[0], range(H2, MT))
        allgather(0, 1)

        # ---- layer 1 aggregation; layer 2 phase A + AG2 fired mid-pipeline ----
        z1T = actT.tile([128, 4, RPAD], mdt, tag="zT")

        def l1_hook(w):
            if w == H2 - 1:
                phase_a(z1T, w_t["W2"], cc_in[1], range(H2))
                allgather(1, 0)
            elif w == MT - 1:
                phase_a(z1T, w_t["W2"], cc_in[1], range(H2, MT))
                allgather(1, 1)

        phase_c(cc_out[0], z1T, b_b["b1"], l1_hook)

        # ---- layer 2 aggregation + head, pipelined per 512-row block ----
        z2T = actT.tile([128, 4, RPAD], mdt, tag="zT")
        z3T = actT.tile([128, 4, RPAD], mdt, tag="zT3")

        def head_block(mb):
            for q in range(4):
                ps = psA.tile([128, D], _f32, tag="psA")
                for k in range(4):
                    nc.tensor.matmul(ps[:], lhsT=w_t["Wf1"][:, k, q * 128:(q + 1) * 128],
                                     rhs=z2T[:, k, mb * 512:(mb + 1) * 512],
                                     start=(k == 0), stop=(k == 3))
                nc.scalar.activation(z3T[:, q, mb * 512:(mb + 1) * 512], ps[:],
                                     ACT.Relu, bias=bf1_t[:, q:q + 1])
            for m in range(mb * 4, (mb + 1) * 4):
                ps2 = psT.tile([128, D_OUT], _f32, tag="psT")
                for k in range(4):
                    nc.tensor.matmul(ps2[:], lhsT=z3T[:, k, m * 128:(m + 1) * 128],
                                     rhs=wf2_t[:, k, :], start=(k == 0), stop=(k == 3))
                lg = work.tile([128, D_OUT], _f32, tag="lg")
                nc.vector.tensor_tensor(lg[:], ps2[:], bf2_b[:], op=ALU.add)
                nmx = work.tile([128, 1], _f32, tag="nmx")
                nc.vector.tensor_reduce(nmx[:], lg[:], axis=mybir.AxisListType.X,
                                        op=ALU.max, negate=True)
                ex = work.tile([128, D_OUT], _f32, tag="ex")
                sm = work.tile([128, 1], _f32, tag="sm")
                nc.scalar.activation(ex[:], lg[:], ACT.Exp, bias=nmx[:, :1], scale=1.0,
                                     accum_out=sm[:, :1])
                rin = work.tile([128, 1], _f32, tag="rin")
                nc.vector.reciprocal(rin[:], sm[:])
                ot = work.tile([128, D_OUT], _f32, tag="ot")
                nc.vector.tensor_scalar_mul(ot[:], ex[:], rin[:, :1])
                nc.sync.dma_start(out_d.ap()[m * 128:(m + 1) * 128, :], ot[:])

        def l2_hook(w):
            if w % 4 == 3:
                head_block(w // 4)

        phase_c(cc_out[1], z2T, b_b["b2"], l2_hook)

    nc.compile()
    return nc


def _run(inputs, trace=False):
    x = np.asarray(inputs["x"], dtype=np.float32)
    edge_index = np.asarray(inputs["edge_index"])
    deg = np.bincount(
        np.concatenate([edge_index[1], np.arange(N, dtype=edge_index.dtype)]),
        minlength=N,
    ).astype(np.float32)
    dinv = np.zeros(N, dtype=np.float32)
    nz = deg > 0
    dinv[nz] = (1.0 / np.sqrt(deg[nz])).astype(np.float32)

    per_core, group_sizes, chunk_base, TC = _prepare(edge_index, dinv)
    nc = _build(group_sizes, chunk_base, TC)

    if MODE == "bf16":
        import ml_dtypes
        mnp = ml_dtypes.bfloat16
    else:
        mnp = np.float32

    in_maps = []
    for c in range(NCORES):
        xp = np.zeros((RPAD, D), dtype=np.float32)
        xp[:RPC] = x[c * RPC:(c + 1) * RPC]
        dv = np.zeros(RPAD, dtype=np.float32)
        dv[:RPC] = dinv[c * RPC:(c + 1) * RPC]
        bb = {k: np.broadcast_to(np.asarray(inputs[k], np.float32), (128, D)).copy()
              for k in ("b1", "b2")}
        in_maps.append({
            "xT": np.ascontiguousarray(xp.T).astype(mnp),
            "dinv": dv,
            "W1": np.asarray(inputs["W1"], np.float32).astype(mnp),
            "W2": np.asarray(inputs["W2"], np.float32).astype(mnp),
            "Wf1": np.asarray(inputs["Wf1"], np.float32).astype(mnp),
            "Wf2": np.asarray(inputs["Wf2"], np.float32).astype(mnp),
            "b1": bb["b1"],
            "b2": bb["b2"],
            "bf1": np.asarray(inputs["bf1"], np.float32),
            "bf2": np.broadcast_to(np.asarray(inputs["bf2"], np.float32),
                                   (128, D_OUT)).copy(),
            "idx": per_core[c]["idx"],
            "wsl": per_core[c]["wsl"],
        })

    res = run_bass_kernel_spmd(nc, in_maps, core_ids=list(range(NCORES)),
                               trace=trace)
    out = np.concatenate([res.results[c]["out"][:RPC] for c in range(NCORES)], axis=0)
    return out, res


def kernel(**inputs):
    out, _ = _run(inputs, trace=False)
    return out
